# revision 11
# baseline (speedup 1.0000x reference)
"""Trainium2 Bass kernel for nn_MHA: 16-head MHA, B=4, S=2048, IN=1024, D=64.

Sharding: q-row data parallel across 8 cores. Core i handles batch b=i//2,
query rows [half*1024, half*1024+1024) with half=i%2. Each core computes its
disjoint slice of BOTH outputs (out rows and mean-probs rows), so no
collectives and no host-side reduction are needed -- only concatenation.

Per-core pipeline (all layouts chosen so no on-chip transpose of the big
probs tensor is needed except one DMA-xbar transpose pass):
  xT [IN, S] (host-transposed)  --PE-->  qT [feat, q], kT [feat, kpos]
  (feature-on-partition so per-feature bias adds are per-partition),
  v [kpos, feat].
  S = qT.T @ kT per head        (PE, bf16, K=64 row-packed head pairs)
  exp + row-sum in one ACT pass (accum_out), fp16 exp output
  probs = exp * (1/(16*sum))    (DVE tensor_scalar, in-place, fp16)
  out2 += probs                 (DVE tensor_tensor, fp32 accumulator)
  probsT via DMA-xbar transpose (fp16, SBUF->SBUF)
  attnT = v.T @ probsT          (PE, fp16, PSUM-accumulated over kpos)
  out = attnT.T @ (16*Wo)       (PE, fp16; the 1/16 from the probs scaling
                                 cancels against the host-prescaled Wo)
Host fixes afterward: out += bv @ Wo + bo (valid because softmax rows sum
to 1, so probs @ (v + bv) = probs @ v + bv).
"""

import sys

if "/opt/trn_rl_repo" not in sys.path:
    sys.path.insert(0, "/opt/trn_rl_repo")

import numpy as np
import ml_dtypes

import concourse.bass as bass
import concourse.mybir as mybir
import concourse.tile as tile
from concourse import bacc
from concourse.bass_utils import run_bass_kernel_spmd

# Problem dims (hardcoded per contract)
B, S, IN, H, D = 4, 2048, 1024, 16, 64
NCORES = 8
QL = S // 2          # 1024 query rows per core
P = 128              # partitions
NIC = IN // P        # 8 input-feature chunks
NHP = H // 2         # 8 head pairs (one 128-wide feature chunk each)
NKB = S // P         # 16 kpos blocks
NQT = QL // P        # 8 query tiles per core
SM_SCALE = 1.0 / np.sqrt(np.float32(D))  # 0.125

BF16 = mybir.dt.bfloat16
FP16 = mybir.dt.float16
FP32 = mybir.dt.float32

# Module-level knobs (test.py may flip these before calling kernel()).
TRACE = False
LAST_RESULTS = None


def _emit(tc, t):
    """Emit the per-core MHA program. t: dict of dram APs."""
    nc = tc.nc
    from contextlib import ExitStack

    # ---------------- persistent pools (live through the whole kernel) ----
    ctx = ExitStack()
    with ctx:
        pers = ctx.enter_context(tc.tile_pool(name="pers", bufs=1))
        qT_s = pers.tile([P, NHP, QL], BF16, tag="qT")       # 16KB/part
        kT_s = pers.tile([P, NHP, S], BF16, tag="kT")        # 32KB/part
        v_s = pers.tile([P, NKB, IN], FP16, tag="v")         # 32KB/part
        wo_s = pers.tile([P, NHP, IN], FP16, tag="wo")       # 16KB/part
        bq_t = pers.tile([P, NIC], FP32, tag="bq")
        bk_t = pers.tile([P, NIC], FP32, tag="bk")

        nc.sync.dma_start(out=wo_s[:], in_=t["wo"].rearrange("(c p) f -> p c f", p=P))
        nc.sync.dma_start(out=bq_t[:], in_=t["bq2"])
        nc.sync.dma_start(out=bk_t[:], in_=t["bk2"])

        # ---------------- phase 0: projections -----------------------------
        with ExitStack() as p0:
            ph0 = p0.enter_context(tc.tile_pool(name="ph0", bufs=1))
            wbuf = p0.enter_context(tc.tile_pool(name="wbuf", bufs=2))
            pps = p0.enter_context(
                tc.tile_pool(name="proj_ps", bufs=4, space="PSUM")
            )

            xT_s = ph0.tile([P, NIC, S], BF16, tag="xT")     # 32KB/part
            xq_s = ph0.tile([P, NIC, QL], BF16, tag="xq")    # 16KB/part
            nc.sync.dma_start(
                out=xT_s[:], in_=t["xT"].rearrange("(c p) k -> p c k", p=P)
            )
            nc.sync.dma_start(
                out=xq_s[:], in_=t["xq"].rearrange("(c p) k -> p c k", p=P)
            )

            # qT[feat, q] = Wq.T @ xq  (+bq per-partition)
            wq_s = wbuf.tile([P, NIC, IN], BF16, tag="w")
            nc.sync.dma_start(
                out=wq_s[:], in_=t["wq"].rearrange("(c p) f -> p c f", p=P)
            )
            for hp in range(NHP):
                for qc in range(QL // 512):
                    ps = pps.tile([P, 512], FP32, tag="ps")
                    for c in range(NIC):
                        nc.tensor.matmul(
                            ps[:],
                            wq_s[:, c, hp * P : (hp + 1) * P],
                            xq_s[:, c, qc * 512 : (qc + 1) * 512],
                            start=(c == 0),
                            stop=(c == NIC - 1),
                        )
                    nc.vector.tensor_scalar_add(
                        qT_s[:, hp, qc * 512 : (qc + 1) * 512],
                        ps[:],
                        bq_t[:, hp : hp + 1],
                    )

            # kT[feat, kpos] = Wk.T @ xT  (+bk)
            wk_s = wbuf.tile([P, NIC, IN], BF16, tag="w")
            nc.sync.dma_start(
                out=wk_s[:], in_=t["wk"].rearrange("(c p) f -> p c f", p=P)
            )
            for hp in range(NHP):
                for kc in range(S // 512):
                    ps = pps.tile([P, 512], FP32, tag="ps")
                    for c in range(NIC):
                        nc.tensor.matmul(
                            ps[:],
                            wk_s[:, c, hp * P : (hp + 1) * P],
                            xT_s[:, c, kc * 512 : (kc + 1) * 512],
                            start=(c == 0),
                            stop=(c == NIC - 1),
                        )
                    nc.vector.tensor_scalar_add(
                        kT_s[:, hp, kc * 512 : (kc + 1) * 512],
                        ps[:],
                        bk_t[:, hp : hp + 1],
                    )

            # v[kpos, feat] = xT.T(blocks) @ Wv   (bv folded on host)
            wv_s = wbuf.tile([P, NIC, IN], BF16, tag="w")
            nc.sync.dma_start(
                out=wv_s[:], in_=t["wv"].rearrange("(c p) f -> p c f", p=P)
            )
            for kb in range(NKB):
                for fc in range(IN // 512):
                    ps = pps.tile([P, 512], FP32, tag="ps")
                    for c in range(NIC):
                        nc.tensor.matmul(
                            ps[:],
                            xT_s[:, c, kb * P : (kb + 1) * P],
                            wv_s[:, c, fc * 512 : (fc + 1) * 512],
                            start=(c == 0),
                            stop=(c == NIC - 1),
                        )
                    nc.scalar.copy(v_s[:, kb, fc * 512 : (fc + 1) * 512], ps[:])

        # ---------------- attention ----------------------------------------
        # Heads processed in pairs: head 2hp in partitions 0:64, head 2hp+1 in
        # 64:128 of feature-chunk hp. S matmuls are row-group packed
        # (tile_position (0,0)/(64,0)); PV matmuls are col-group packed
        # ((0,0)/(0,64)) -- each pair runs concurrently in the PE array.
        work = ctx.enter_context(tc.tile_pool(name="work", bufs=2))
        o2pool = ctx.enter_context(tc.tile_pool(name="o2", bufs=4))
        small = ctx.enter_context(tc.tile_pool(name="small", bufs=8))
        ptp = ctx.enter_context(tc.tile_pool(name="ptp", bufs=1))
        sps = ctx.enter_context(tc.tile_pool(name="s_ps", bufs=2, space="PSUM"))
        pvps = ctx.enter_context(tc.tile_pool(name="pv_ps", bufs=2, space="PSUM"))
        opps = ctx.enter_context(tc.tile_pool(name="op_ps", bufs=2, space="PSUM"))

        NQB = QL // 512  # 2 query blocks of 512
        for qb in range(NQB):
            out2_acc = [
                o2pool.tile([P, S], FP16, tag="o2", name=f"o2_{qb}_{i}")
                for i in range(4)
            ]
            attnT_s = work.tile([P, NHP, 512], FP16, tag="attnT")

            for hp in range(NHP):
                pT = [
                    ptp.tile([P, NKB, 512], FP16, tag=f"pT{h2}", name=f"pT_{qb}_{hp}_{h2}")
                    for h2 in range(2)
                ]
                for qt in range(4):
                    qg = qb * 4 + qt  # global qtile in this core
                    ex = [
                        work.tile([P, S], FP16, tag=f"exp{h2}", name=f"ex_{qb}_{hp}_{qt}_{h2}")
                        for h2 in range(2)
                    ]
                    sums = [
                        small.tile([P, 2], FP32, tag=f"sm{h2}", name=f"sm_{qb}_{hp}_{qt}_{h2}")
                        for h2 in range(2)
                    ]
                    # scores + exp + row-sums, 1024-wide chunks, head pair
                    # interleaved so the K=64 matmuls pack into row groups.
                    for c in range(2):
                        s_ps = [
                            sps.tile([P, 1024], FP32, tag="sps", name=f"sps_{qb}_{hp}_{qt}_{c}_{h2}")
                            for h2 in range(2)
                        ]
                        for cc in range(2):
                            ks = slice(c * 1024 + cc * 512, c * 1024 + (cc + 1) * 512)
                            for h2 in range(2):
                                pr = h2 * 64
                                nc.tensor.matmul(
                                    s_ps[h2][:, cc * 512 : (cc + 1) * 512],
                                    qT_s[pr : pr + 64, hp, qg * P : (qg + 1) * P],
                                    kT_s[pr : pr + 64, hp, ks],
                                    start=True,
                                    stop=True,
                                    tile_position=(pr, 0),
                                )
                        for h2 in range(2):
                            nc.scalar.activation(
                                ex[h2][:, c * 1024 : (c + 1) * 1024],
                                s_ps[h2][:],
                                mybir.ActivationFunctionType.Exp,
                                scale=float(SM_SCALE),
                                accum_out=sums[h2][:, c : c + 1],
                            )
                    for h2 in range(2):
                        # r = 1/(16*sum): combine chunk sums, scale, invert
                        sum1 = small.tile([P, 1], FP32, tag="s1", name=f"s1_{qb}_{hp}_{qt}_{h2}")
                        r16 = small.tile([P, 1], FP32, tag="r16", name=f"r16_{qb}_{hp}_{qt}_{h2}")
                        nc.vector.reduce_sum(
                            sum1[:], sums[h2][:], axis=mybir.AxisListType.X
                        )
                        nc.vector.tensor_scalar_mul(sum1[:], sum1[:], 16.0)
                        nc.vector.reciprocal(r16[:], sum1[:])
                        # normalize in place: probs/16 (fp16)
                        nc.vector.tensor_scalar_mul(
                            ex[h2][:], ex[h2][:], r16[:, 0:1]
                        )
                    # mean-probs accumulation (fp16 acc; final cast on DMA out)
                    if hp == 0:
                        nc.vector.tensor_copy(out2_acc[qt][:], ex[0][:])
                    else:
                        nc.vector.tensor_add(
                            out2_acc[qt][:], out2_acc[qt][:], ex[0][:]
                        )
                    nc.vector.tensor_add(
                        out2_acc[qt][:], out2_acc[qt][:], ex[1][:]
                    )
                    # transpose probs -> [kpos, q] layout for PV
                    for h2 in range(2):
                        nc.sync.dma_start_transpose(
                            out=pT[h2][:, :, qt * P : (qt + 1) * P],
                            in_=ex[h2][:],
                        )
                # PV: attnT[d, q] accumulated over kpos blocks; col-packed pair
                pv = pvps.tile([P, 512], FP32, tag="pv", name=f"pv_{qb}_{hp}")
                for kc in range(NKB):
                    for h2 in range(2):
                        h = hp * 2 + h2
                        pr = h2 * 64
                        nc.tensor.matmul(
                            pv[pr : pr + 64, :],
                            v_s[:, kc, h * 64 : (h + 1) * 64],
                            pT[h2][:, kc, :],
                            start=(kc == 0),
                            stop=(kc == NKB - 1),
                            tile_position=(0, pr),
                        )
                nc.scalar.copy(attnT_s[:, hp, :], pv[:])

            # output projection for this q-block
            for qt in range(4):
                qg = qb * 4 + qt
                ostage = work.tile([P, IN], FP32, tag="ostage", name=f"ost_{qb}_{qt}")
                for oc in range(IN // 512):
                    ops = opps.tile([P, 512], FP32, tag="ops", name=f"ops_{qb}_{qt}_{oc}")
                    for hp in range(NHP):
                        nc.tensor.matmul(
                            ops[:],
                            attnT_s[:, hp, qt * P : (qt + 1) * P],
                            wo_s[:, hp, oc * 512 : (oc + 1) * 512],
                            start=(hp == 0),
                            stop=(hp == NHP - 1),
                        )
                    nc.scalar.copy(ostage[:, oc * 512 : (oc + 1) * 512], ops[:])
                nc.sync.dma_start(
                    out=t["out_sl"][qg * P : (qg + 1) * P, :], in_=ostage[:]
                )
                # SWDGE DMA casts the fp16 accumulator to the f32 output
                nc.gpsimd.dma_start(
                    out=t["out2_sl"][qg * P : (qg + 1) * P, :], in_=out2_acc[qt][:]
                )


_BUILT = {}


def _build(repeats=1):
    if repeats in _BUILT:
        return _BUILT[repeats]
    # Bacc (not raw Bass): its compile() pass splits multi-semaphore waits
    # into EventSemaphore pre-waits -- walrus codegen allows only 1 wait per
    # compute instruction.
    nc = bacc.Bacc("TRN2", target_bir_lowering=False, debug=False)
    t = {}
    t["xT"] = nc.dram_tensor("xT", [IN, S], BF16, kind="ExternalInput").ap()
    t["xq"] = nc.dram_tensor("xq", [IN, QL], BF16, kind="ExternalInput").ap()
    t["wq"] = nc.dram_tensor("wq", [IN, IN], BF16, kind="ExternalInput").ap()
    t["wk"] = nc.dram_tensor("wk", [IN, IN], BF16, kind="ExternalInput").ap()
    t["wv"] = nc.dram_tensor("wv", [IN, IN], BF16, kind="ExternalInput").ap()
    t["wo"] = nc.dram_tensor("wo", [IN, IN], FP16, kind="ExternalInput").ap()
    t["bq2"] = nc.dram_tensor("bq2", [P, NIC], FP32, kind="ExternalInput").ap()
    t["bk2"] = nc.dram_tensor("bk2", [P, NIC], FP32, kind="ExternalInput").ap()
    t["out_sl"] = nc.dram_tensor("out_sl", [QL, IN], FP32, kind="ExternalOutput").ap()
    t["out2_sl"] = nc.dram_tensor("out2_sl", [QL, S], FP32, kind="ExternalOutput").ap()

    with tile.TileContext(nc) as tc:
        for _ in range(repeats):  # repeats>1: timing builds only
            _emit(tc, t)
    nc.compile()
    _BUILT[repeats] = nc
    return nc


def _host_prep(x, Wq, Wk, Wv, Wo, bq, bk):
    """Build the 8 per-core input maps."""
    bf = ml_dtypes.bfloat16
    wq_b = Wq.astype(bf)
    wk_b = Wk.astype(bf)
    wv_b = Wv.astype(bf)
    wo_h = (Wo * np.float32(16.0)).astype(np.float16)
    bq2 = np.ascontiguousarray(bq.reshape(NIC, P).T.astype(np.float32))
    bk2 = np.ascontiguousarray(bk.reshape(NIC, P).T.astype(np.float32))
    in_maps = []
    for b in range(B):
        xT_b = x[b].T.astype(bf)  # [IN, S], contiguous via astype copy
        for half in range(2):
            xq_b = np.ascontiguousarray(xT_b[:, half * QL : (half + 1) * QL])
            in_maps.append(
                {
                    "xT": xT_b,
                    "xq": xq_b,
                    "wq": wq_b,
                    "wk": wk_b,
                    "wv": wv_b,
                    "wo": wo_h,
                    "bq2": bq2,
                    "bk2": bk2,
                }
            )
    return in_maps


def kernel(x, Wq, bq, Wk, bk, Wv, bv, Wo, bo):
    global LAST_RESULTS
    x = np.asarray(x, dtype=np.float32)
    Wq = np.asarray(Wq, dtype=np.float32)
    Wk = np.asarray(Wk, dtype=np.float32)
    Wv = np.asarray(Wv, dtype=np.float32)
    Wo = np.asarray(Wo, dtype=np.float32)
    bq = np.asarray(bq, dtype=np.float32)
    bk = np.asarray(bk, dtype=np.float32)
    bv = np.asarray(bv, dtype=np.float32)
    bo = np.asarray(bo, dtype=np.float32)

    nc = _build()
    in_maps = _host_prep(x, Wq, Wk, Wv, Wo, bq, bk)
    # trace/NTFF profiling is unavailable in this container (no axon.trn
    # hook); run_bass_kernel_spmd with trace=False goes straight to PJRT.
    res = run_bass_kernel_spmd(
        nc, in_maps, core_ids=list(range(NCORES)), trace=False
    )
    LAST_RESULTS = res

    out = np.empty((B, S, IN), dtype=np.float32)
    out2 = np.empty((B, S, S), dtype=np.float32)
    for i in range(NCORES):
        b, half = i // 2, i % 2
        rows = slice(half * QL, (half + 1) * QL)
        out[b, rows, :] = res.results[i]["out_sl"]
        out2[b, rows, :] = res.results[i]["out2_sl"]

    # host bias fixes: probs rows sum to 1 -> attn bias = bv @ Wo; plus bo.
    out += (bv @ Wo + bo)[None, None, :]
    return out, out2


# revision 13
# speedup vs baseline: 124.0020x; 124.0020x over previous
"""Trainium2 Bass kernel for nn_MHA: 16-head MHA, B=4, S=2048, IN=1024, D=64.

Sharding: q-row data parallel across 8 cores. Core i handles batch b=i//2,
query rows [half*1024, half*1024+1024) with half=i%2. Each core computes its
disjoint slice of BOTH outputs (out rows and mean-probs rows), so no
collectives and no host-side reduction are needed -- only concatenation.

Per-core pipeline (all layouts chosen so no on-chip transpose of the big
probs tensor is needed except one DMA-xbar transpose pass):
  xT [IN, S] (host-transposed)  --PE-->  qT [feat, q], kT [feat, kpos]
  (feature-on-partition so per-feature bias adds are per-partition),
  v [kpos, feat].
  S = qT.T @ kT per head        (PE, bf16, K=64 row-packed head pairs)
  exp + row-sum in one ACT pass (accum_out), fp16 exp output
  probs = exp * (1/(16*sum))    (DVE tensor_scalar, in-place, fp16)
  out2 += probs                 (DVE tensor_tensor, fp32 accumulator)
  probsT via DMA-xbar transpose (fp16, SBUF->SBUF)
  attnT = v.T @ probsT          (PE, fp16, PSUM-accumulated over kpos)
  out = attnT.T @ (16*Wo)       (PE, fp16; the 1/16 from the probs scaling
                                 cancels against the host-prescaled Wo)
Host fixes afterward: out += bv @ Wo + bo (valid because softmax rows sum
to 1, so probs @ (v + bv) = probs @ v + bv).
"""

import sys

if "/opt/trn_rl_repo" not in sys.path:
    sys.path.insert(0, "/opt/trn_rl_repo")

import numpy as np
import ml_dtypes

import concourse.bass as bass
import concourse.mybir as mybir
import concourse.tile as tile
from concourse import bacc
from concourse.bass_utils import run_bass_kernel_spmd

# Problem dims (hardcoded per contract)
B, S, IN, H, D = 4, 2048, 1024, 16, 64
NCORES = 8
QL = S // 2          # 1024 query rows per core
P = 128              # partitions
NIC = IN // P        # 8 input-feature chunks
NHP = H // 2         # 8 head pairs (one 128-wide feature chunk each)
NKB = S // P         # 16 kpos blocks
NQT = QL // P        # 8 query tiles per core
SM_SCALE = 1.0 / np.sqrt(np.float32(D))  # 0.125

BF16 = mybir.dt.bfloat16
FP16 = mybir.dt.float16
FP32 = mybir.dt.float32

# Module-level knobs (test.py may flip these before calling kernel()).
TRACE = False
LAST_RESULTS = None


def _emit(tc, t):
    """Emit the per-core MHA program. t: dict of dram APs."""
    nc = tc.nc
    from contextlib import ExitStack

    # ---------------- persistent pools (live through the whole kernel) ----
    ctx = ExitStack()
    with ctx:
        pers = ctx.enter_context(tc.tile_pool(name="pers", bufs=1))
        qT_s = pers.tile([P, NHP, QL], BF16, tag="qT")       # 16KB/part
        kT_s = pers.tile([P, NHP, S], BF16, tag="kT")        # 32KB/part
        v_s = pers.tile([P, NKB, IN], FP16, tag="v")         # 32KB/part
        wo_s = pers.tile([P, NHP, IN], FP16, tag="wo")       # 16KB/part
        bq_t = pers.tile([P, NIC], FP32, tag="bq")
        bk_t = pers.tile([P, NIC], FP32, tag="bk")

        nc.sync.dma_start(out=wo_s[:], in_=t["wo"].rearrange("(c p) f -> p c f", p=P))
        nc.sync.dma_start(out=bq_t[:], in_=t["bq2"])
        nc.sync.dma_start(out=bk_t[:], in_=t["bk2"])

        # ---------------- phase 0: projections -----------------------------
        with ExitStack() as p0:
            ph0 = p0.enter_context(tc.tile_pool(name="ph0", bufs=1))
            wbuf = p0.enter_context(tc.tile_pool(name="wbuf", bufs=2))
            pps = p0.enter_context(
                tc.tile_pool(name="proj_ps", bufs=4, space="PSUM")
            )

            xT_s = ph0.tile([P, NIC, S], BF16, tag="xT")     # 32KB/part
            xq_s = ph0.tile([P, NIC, QL], BF16, tag="xq")    # 16KB/part
            nc.sync.dma_start(
                out=xT_s[:], in_=t["xT"].rearrange("(c p) k -> p c k", p=P)
            )
            nc.sync.dma_start(
                out=xq_s[:], in_=t["xq"].rearrange("(c p) k -> p c k", p=P)
            )

            # qT[feat, q] = Wq.T @ xq  (+bq per-partition)
            wq_s = wbuf.tile([P, NIC, IN], BF16, tag="w")
            nc.sync.dma_start(
                out=wq_s[:], in_=t["wq"].rearrange("(c p) f -> p c f", p=P)
            )
            for hp in range(NHP):
                for qc in range(QL // 512):
                    ps = pps.tile([P, 512], FP32, tag="ps")
                    for c in range(NIC):
                        nc.tensor.matmul(
                            ps[:],
                            wq_s[:, c, hp * P : (hp + 1) * P],
                            xq_s[:, c, qc * 512 : (qc + 1) * 512],
                            start=(c == 0),
                            stop=(c == NIC - 1),
                        )
                    nc.vector.tensor_scalar_add(
                        qT_s[:, hp, qc * 512 : (qc + 1) * 512],
                        ps[:],
                        bq_t[:, hp : hp + 1],
                    )

            # kT[feat, kpos] = Wk.T @ xT  (+bk)
            wk_s = wbuf.tile([P, NIC, IN], BF16, tag="w")
            nc.sync.dma_start(
                out=wk_s[:], in_=t["wk"].rearrange("(c p) f -> p c f", p=P)
            )
            for hp in range(NHP):
                for kc in range(S // 512):
                    ps = pps.tile([P, 512], FP32, tag="ps")
                    for c in range(NIC):
                        nc.tensor.matmul(
                            ps[:],
                            wk_s[:, c, hp * P : (hp + 1) * P],
                            xT_s[:, c, kc * 512 : (kc + 1) * 512],
                            start=(c == 0),
                            stop=(c == NIC - 1),
                        )
                    nc.vector.tensor_scalar_add(
                        kT_s[:, hp, kc * 512 : (kc + 1) * 512],
                        ps[:],
                        bk_t[:, hp : hp + 1],
                    )

            # v[kpos, feat] = xT.T(blocks) @ Wv   (bv folded on host)
            wv_s = wbuf.tile([P, NIC, IN], BF16, tag="w")
            nc.sync.dma_start(
                out=wv_s[:], in_=t["wv"].rearrange("(c p) f -> p c f", p=P)
            )
            for kb in range(NKB):
                for fc in range(IN // 512):
                    ps = pps.tile([P, 512], FP32, tag="ps")
                    for c in range(NIC):
                        nc.tensor.matmul(
                            ps[:],
                            xT_s[:, c, kb * P : (kb + 1) * P],
                            wv_s[:, c, fc * 512 : (fc + 1) * 512],
                            start=(c == 0),
                            stop=(c == NIC - 1),
                        )
                    nc.scalar.copy(v_s[:, kb, fc * 512 : (fc + 1) * 512], ps[:])

        # ---------------- attention ----------------------------------------
        # Heads processed in pairs: head 2hp in partitions 0:64, head 2hp+1 in
        # 64:128 of feature-chunk hp. S matmuls are row-group packed
        # (tile_position (0,0)/(64,0)); PV matmuls are col-group packed
        # ((0,0)/(0,64)) -- each pair runs concurrently in the PE array.
        work = ctx.enter_context(tc.tile_pool(name="work", bufs=2))
        o2pool = ctx.enter_context(tc.tile_pool(name="o2", bufs=4))
        small = ctx.enter_context(tc.tile_pool(name="small", bufs=8))
        ptp = ctx.enter_context(tc.tile_pool(name="ptp", bufs=1))
        sps = ctx.enter_context(tc.tile_pool(name="s_ps", bufs=2, space="PSUM"))
        pvps = ctx.enter_context(tc.tile_pool(name="pv_ps", bufs=1, space="PSUM"))
        opps = ctx.enter_context(tc.tile_pool(name="op_ps", bufs=2, space="PSUM"))

        NQB = QL // 512  # 2 query blocks of 512
        for qb in range(NQB):
            out2_acc = [
                o2pool.tile([P, S], FP16, tag="o2", name=f"o2_{qb}_{i}")
                for i in range(4)
            ]
            attnT_s = work.tile([P, NHP, 512], FP16, tag="attnT")

            for hp in range(NHP):
                pT = [
                    ptp.tile([P, NKB, 512], FP16, tag=f"pT{h2}", name=f"pT_{qb}_{hp}_{h2}")
                    for h2 in range(2)
                ]
                for qt in range(4):
                    qg = qb * 4 + qt  # global qtile in this core
                    ex = [
                        work.tile([P, S], FP16, tag=f"exp{h2}", name=f"ex_{qb}_{hp}_{qt}_{h2}")
                        for h2 in range(2)
                    ]
                    sums = [
                        small.tile([P, 2], FP32, tag=f"sm{h2}", name=f"sm_{qb}_{hp}_{qt}_{h2}")
                        for h2 in range(2)
                    ]
                    # scores + exp + row-sums, 1024-wide chunks, head pair
                    # interleaved so the K=64 matmuls pack into row groups.
                    for c in range(2):
                        s_ps = [
                            sps.tile([P, 1024], FP32, tag="sps", name=f"sps_{qb}_{hp}_{qt}_{c}_{h2}")
                            for h2 in range(2)
                        ]
                        for cc in range(2):
                            ks = slice(c * 1024 + cc * 512, c * 1024 + (cc + 1) * 512)
                            for h2 in range(2):
                                pr = h2 * 64
                                nc.tensor.matmul(
                                    s_ps[h2][:, cc * 512 : (cc + 1) * 512],
                                    qT_s[pr : pr + 64, hp, qg * P : (qg + 1) * P],
                                    kT_s[pr : pr + 64, hp, ks],
                                    start=True,
                                    stop=True,
                                    tile_position=(pr, 0),
                                )
                        for h2 in range(2):
                            nc.scalar.activation(
                                ex[h2][:, c * 1024 : (c + 1) * 1024],
                                s_ps[h2][:],
                                mybir.ActivationFunctionType.Exp,
                                scale=float(SM_SCALE),
                                accum_out=sums[h2][:, c : c + 1],
                            )
                    for h2 in range(2):
                        # r = 1/(16*sum): combine chunk sums, scale, invert
                        sum1 = small.tile([P, 1], FP32, tag="s1", name=f"s1_{qb}_{hp}_{qt}_{h2}")
                        r16 = small.tile([P, 1], FP32, tag="r16", name=f"r16_{qb}_{hp}_{qt}_{h2}")
                        nc.vector.reduce_sum(
                            sum1[:], sums[h2][:], axis=mybir.AxisListType.X
                        )
                        nc.vector.tensor_scalar_mul(sum1[:], sum1[:], 16.0)
                        nc.vector.reciprocal(r16[:], sum1[:])
                        # normalize in place: probs/16 (fp16)
                        nc.vector.tensor_scalar_mul(
                            ex[h2][:], ex[h2][:], r16[:, 0:1]
                        )
                    # mean-probs accumulation (fp16 acc; final cast on DMA out)
                    if hp == 0:
                        nc.vector.tensor_copy(out2_acc[qt][:], ex[0][:])
                    else:
                        nc.vector.tensor_add(
                            out2_acc[qt][:], out2_acc[qt][:], ex[0][:]
                        )
                    nc.vector.tensor_add(
                        out2_acc[qt][:], out2_acc[qt][:], ex[1][:]
                    )
                    # transpose probs -> [kpos, q] layout for PV
                    for h2 in range(2):
                        nc.sync.dma_start_transpose(
                            out=pT[h2][:, :, qt * P : (qt + 1) * P],
                            in_=ex[h2][:],
                        )
                # PV: attnT[d, q] accumulated over kpos blocks; col-packed pair.
                # Separate PSUM banks per head: an accumulation group's
                # start=True clears has_written for its whole bank, so two
                # interleaved groups cannot share one bank.
                pv = [
                    pvps.tile([P, 512], FP32, tag=f"pv{h2}", name=f"pv_{qb}_{hp}_{h2}")
                    for h2 in range(2)
                ]
                for kc in range(NKB):
                    for h2 in range(2):
                        h = hp * 2 + h2
                        pr = h2 * 64
                        nc.tensor.matmul(
                            pv[h2][pr : pr + 64, :],
                            v_s[:, kc, h * 64 : (h + 1) * 64],
                            pT[h2][:, kc, :],
                            start=(kc == 0),
                            stop=(kc == NKB - 1),
                            tile_position=(0, pr),
                        )
                for h2 in range(2):
                    pr = h2 * 64
                    nc.scalar.copy(
                        attnT_s[pr : pr + 64, hp, :], pv[h2][pr : pr + 64, :]
                    )

            # output projection for this q-block
            for qt in range(4):
                qg = qb * 4 + qt
                ostage = work.tile([P, IN], FP32, tag="ostage", name=f"ost_{qb}_{qt}")
                for oc in range(IN // 512):
                    ops = opps.tile([P, 512], FP32, tag="ops", name=f"ops_{qb}_{qt}_{oc}")
                    for hp in range(NHP):
                        nc.tensor.matmul(
                            ops[:],
                            attnT_s[:, hp, qt * P : (qt + 1) * P],
                            wo_s[:, hp, oc * 512 : (oc + 1) * 512],
                            start=(hp == 0),
                            stop=(hp == NHP - 1),
                        )
                    nc.scalar.copy(ostage[:, oc * 512 : (oc + 1) * 512], ops[:])
                nc.sync.dma_start(
                    out=t["out_sl"][qg * P : (qg + 1) * P, :], in_=ostage[:]
                )
                # SWDGE DMA casts the fp16 accumulator to the f32 output
                nc.gpsimd.dma_start(
                    out=t["out2_sl"][qg * P : (qg + 1) * P, :], in_=out2_acc[qt][:]
                )


_BUILT = {}


def _build(repeats=1):
    if repeats in _BUILT:
        return _BUILT[repeats]
    # Bacc (not raw Bass): its compile() pass splits multi-semaphore waits
    # into EventSemaphore pre-waits -- walrus codegen allows only 1 wait per
    # compute instruction.
    nc = bacc.Bacc("TRN2", target_bir_lowering=False, debug=False)
    t = {}
    t["xT"] = nc.dram_tensor("xT", [IN, S], BF16, kind="ExternalInput").ap()
    t["xq"] = nc.dram_tensor("xq", [IN, QL], BF16, kind="ExternalInput").ap()
    t["wq"] = nc.dram_tensor("wq", [IN, IN], BF16, kind="ExternalInput").ap()
    t["wk"] = nc.dram_tensor("wk", [IN, IN], BF16, kind="ExternalInput").ap()
    t["wv"] = nc.dram_tensor("wv", [IN, IN], BF16, kind="ExternalInput").ap()
    t["wo"] = nc.dram_tensor("wo", [IN, IN], FP16, kind="ExternalInput").ap()
    t["bq2"] = nc.dram_tensor("bq2", [P, NIC], FP32, kind="ExternalInput").ap()
    t["bk2"] = nc.dram_tensor("bk2", [P, NIC], FP32, kind="ExternalInput").ap()
    t["out_sl"] = nc.dram_tensor("out_sl", [QL, IN], FP32, kind="ExternalOutput").ap()
    t["out2_sl"] = nc.dram_tensor("out2_sl", [QL, S], FP32, kind="ExternalOutput").ap()

    with tile.TileContext(nc) as tc:
        for _ in range(repeats):  # repeats>1: timing builds only
            _emit(tc, t)
    nc.compile()
    _BUILT[repeats] = nc
    return nc


def _host_prep(x, Wq, Wk, Wv, Wo, bq, bk):
    """Build the 8 per-core input maps."""
    bf = ml_dtypes.bfloat16
    wq_b = Wq.astype(bf)
    wk_b = Wk.astype(bf)
    wv_b = Wv.astype(bf)
    wo_h = (Wo * np.float32(16.0)).astype(np.float16)
    bq2 = np.ascontiguousarray(bq.reshape(NIC, P).T.astype(np.float32))
    bk2 = np.ascontiguousarray(bk.reshape(NIC, P).T.astype(np.float32))
    in_maps = []
    for b in range(B):
        xT_b = x[b].T.astype(bf)  # [IN, S], contiguous via astype copy
        for half in range(2):
            xq_b = np.ascontiguousarray(xT_b[:, half * QL : (half + 1) * QL])
            in_maps.append(
                {
                    "xT": xT_b,
                    "xq": xq_b,
                    "wq": wq_b,
                    "wk": wk_b,
                    "wv": wv_b,
                    "wo": wo_h,
                    "bq2": bq2,
                    "bk2": bk2,
                }
            )
    return in_maps


def kernel(x, Wq, bq, Wk, bk, Wv, bv, Wo, bo):
    global LAST_RESULTS
    x = np.asarray(x, dtype=np.float32)
    Wq = np.asarray(Wq, dtype=np.float32)
    Wk = np.asarray(Wk, dtype=np.float32)
    Wv = np.asarray(Wv, dtype=np.float32)
    Wo = np.asarray(Wo, dtype=np.float32)
    bq = np.asarray(bq, dtype=np.float32)
    bk = np.asarray(bk, dtype=np.float32)
    bv = np.asarray(bv, dtype=np.float32)
    bo = np.asarray(bo, dtype=np.float32)

    nc = _build()
    in_maps = _host_prep(x, Wq, Wk, Wv, Wo, bq, bk)
    # trace/NTFF profiling is unavailable in this container (no axon.trn
    # hook); run_bass_kernel_spmd with trace=False goes straight to PJRT.
    res = run_bass_kernel_spmd(
        nc, in_maps, core_ids=list(range(NCORES)), trace=False
    )
    LAST_RESULTS = res

    out = np.empty((B, S, IN), dtype=np.float32)
    out2 = np.empty((B, S, S), dtype=np.float32)
    for i in range(NCORES):
        b, half = i // 2, i % 2
        rows = slice(half * QL, (half + 1) * QL)
        out[b, rows, :] = res.results[i]["out_sl"]
        out2[b, rows, :] = res.results[i]["out2_sl"]

    # host bias fixes: probs rows sum to 1 -> attn bias = bv @ Wo; plus bo.
    out += (bv @ Wo + bo)[None, None, :]
    return out, out2


# revision 17
# speedup vs baseline: 150.2022x; 1.2113x over previous
"""Trainium2 Bass kernel for nn_MHA: 16-head MHA, B=4, S=2048, IN=1024, D=64.

Sharding: q-row data parallel across 8 cores. Core i handles batch b=i//2,
query rows [half*1024, half*1024+1024) with half=i%2. Each core computes its
disjoint slice of BOTH outputs (out rows and mean-probs rows), so no
collectives and no host-side reduction are needed -- only concatenation.

Per-core pipeline (all layouts chosen so no on-chip transpose of the big
probs tensor is needed except one DMA-xbar transpose pass):
  xT [IN, S] (host-transposed)  --PE-->  qT [feat, q], kT [feat, kpos]
  (feature-on-partition so per-feature bias adds are per-partition),
  v [kpos, feat].
  S = qT.T @ kT per head        (PE, bf16, K=64 row-packed head pairs)
  exp + row-sum in one ACT pass (accum_out), fp16 exp output
  probs = exp * (1/(16*sum))    (DVE tensor_scalar, in-place, fp16)
  out2 += probs                 (DVE tensor_tensor, fp32 accumulator)
  probsT via DMA-xbar transpose (fp16, SBUF->SBUF)
  attnT = v.T @ probsT          (PE, fp16, PSUM-accumulated over kpos)
  out = attnT.T @ (16*Wo)       (PE, fp16; the 1/16 from the probs scaling
                                 cancels against the host-prescaled Wo)
Host fixes afterward: out += bv @ Wo + bo (valid because softmax rows sum
to 1, so probs @ (v + bv) = probs @ v + bv).
"""

import sys

if "/opt/trn_rl_repo" not in sys.path:
    sys.path.insert(0, "/opt/trn_rl_repo")

import numpy as np
import ml_dtypes

import concourse.bass as bass
import concourse.mybir as mybir
import concourse.tile as tile
from concourse import bacc
from concourse.bass_utils import run_bass_kernel_spmd

# Problem dims (hardcoded per contract)
B, S, IN, H, D = 4, 2048, 1024, 16, 64
NCORES = 8
QL = S // 2          # 1024 query rows per core
P = 128              # partitions
NIC = IN // P        # 8 input-feature chunks
NHP = H // 2         # 8 head pairs (one 128-wide feature chunk each)
NKB = S // P         # 16 kpos blocks
NQT = QL // P        # 8 query tiles per core
SM_SCALE = 1.0 / np.sqrt(np.float32(D))  # 0.125

BF16 = mybir.dt.bfloat16
FP16 = mybir.dt.float16
FP32 = mybir.dt.float32

# Module-level knobs (test.py may flip these before calling kernel()).
TRACE = False
LAST_RESULTS = None
# Timing-ablation flags (dev only; breaks numerics): set of strings among
# {"notrans", "nott", "nopv", "nos", "noexp", "noproj"}.
ABLATE = frozenset()


def _emit(tc, t):
    """Emit the per-core MHA program. t: dict of dram APs."""
    nc = tc.nc
    from contextlib import ExitStack

    # ---------------- persistent pools (live through the whole kernel) ----
    ctx = ExitStack()
    with ctx:
        pers = ctx.enter_context(tc.tile_pool(name="pers", bufs=1))
        qT_s = pers.tile([P, NHP, QL], BF16, tag="qT")       # 16KB/part
        kT_s = pers.tile([P, NHP, S], BF16, tag="kT")        # 32KB/part
        v_s = pers.tile([P, NKB, IN], FP16, tag="v")         # 32KB/part
        wo_s = pers.tile([P, NHP, IN], FP16, tag="wo")       # 16KB/part
        bq_t = pers.tile([P, NIC], FP32, tag="bq")
        bk_t = pers.tile([P, NIC], FP32, tag="bk")

        nc.sync.dma_start(out=wo_s[:], in_=t["wo"].rearrange("(c p) f -> p c f", p=P))
        nc.sync.dma_start(out=bq_t[:], in_=t["bq2"])
        nc.sync.dma_start(out=bk_t[:], in_=t["bk2"])

        # ---------------- phase 0: projections -----------------------------
        with ExitStack() as p0:
            ph0 = p0.enter_context(tc.tile_pool(name="ph0", bufs=1))
            wbuf = p0.enter_context(tc.tile_pool(name="wbuf", bufs=2))
            pps = p0.enter_context(
                tc.tile_pool(name="proj_ps", bufs=4, space="PSUM")
            )

            xT_s = ph0.tile([P, NIC, S], BF16, tag="xT")     # 32KB/part
            xq_s = ph0.tile([P, NIC, QL], BF16, tag="xq")    # 16KB/part
            nc.sync.dma_start(
                out=xT_s[:], in_=t["xT"].rearrange("(c p) k -> p c k", p=P)
            )
            nc.sync.dma_start(
                out=xq_s[:], in_=t["xq"].rearrange("(c p) k -> p c k", p=P)
            )

            # qT[feat, q] = Wq.T @ xq  (+bq per-partition)
            wq_s = wbuf.tile([P, NIC, IN], BF16, tag="w")
            nc.sync.dma_start(
                out=wq_s[:], in_=t["wq"].rearrange("(c p) f -> p c f", p=P)
            )
            for hp in range(NHP):
                for qc in range(QL // 512):
                    ps = pps.tile([P, 512], FP32, tag="ps")
                    for c in range(NIC):
                        nc.tensor.matmul(
                            ps[:],
                            wq_s[:, c, hp * P : (hp + 1) * P],
                            xq_s[:, c, qc * 512 : (qc + 1) * 512],
                            start=(c == 0),
                            stop=(c == NIC - 1),
                        )
                    nc.vector.tensor_scalar_add(
                        qT_s[:, hp, qc * 512 : (qc + 1) * 512],
                        ps[:],
                        bq_t[:, hp : hp + 1],
                    )

            # kT[feat, kpos] = Wk.T @ xT  (+bk)
            wk_s = wbuf.tile([P, NIC, IN], BF16, tag="w")
            nc.sync.dma_start(
                out=wk_s[:], in_=t["wk"].rearrange("(c p) f -> p c f", p=P)
            )
            for hp in range(NHP):
                for kc in range(S // 512):
                    ps = pps.tile([P, 512], FP32, tag="ps")
                    for c in range(NIC):
                        nc.tensor.matmul(
                            ps[:],
                            wk_s[:, c, hp * P : (hp + 1) * P],
                            xT_s[:, c, kc * 512 : (kc + 1) * 512],
                            start=(c == 0),
                            stop=(c == NIC - 1),
                        )
                    nc.vector.tensor_scalar_add(
                        kT_s[:, hp, kc * 512 : (kc + 1) * 512],
                        ps[:],
                        bk_t[:, hp : hp + 1],
                    )

            # v[kpos, feat] = xT.T(blocks) @ Wv   (bv folded on host)
            wv_s = wbuf.tile([P, NIC, IN], BF16, tag="w")
            nc.sync.dma_start(
                out=wv_s[:], in_=t["wv"].rearrange("(c p) f -> p c f", p=P)
            )
            for kb in range(NKB):
                for fc in range(IN // 512):
                    ps = pps.tile([P, 512], FP32, tag="ps")
                    for c in range(NIC):
                        nc.tensor.matmul(
                            ps[:],
                            xT_s[:, c, kb * P : (kb + 1) * P],
                            wv_s[:, c, fc * 512 : (fc + 1) * 512],
                            start=(c == 0),
                            stop=(c == NIC - 1),
                        )
                    nc.scalar.copy(v_s[:, kb, fc * 512 : (fc + 1) * 512], ps[:])

        # ---------------- attention ----------------------------------------
        # Heads processed in pairs: head 2hp in partitions 0:64, head 2hp+1 in
        # 64:128 of feature-chunk hp. S matmuls are row-group packed
        # (tile_position (0,0)/(64,0)); PV matmuls are col-group packed
        # ((0,0)/(0,64)) -- each pair runs concurrently in the PE array.
        work = ctx.enter_context(tc.tile_pool(name="work", bufs=2))
        o2pool = ctx.enter_context(tc.tile_pool(name="o2", bufs=4))
        small = ctx.enter_context(tc.tile_pool(name="small", bufs=8))
        ptp = ctx.enter_context(tc.tile_pool(name="ptp", bufs=1))
        sps = ctx.enter_context(tc.tile_pool(name="s_ps", bufs=2, space="PSUM"))
        pvps = ctx.enter_context(tc.tile_pool(name="pv_ps", bufs=1, space="PSUM"))
        opps = ctx.enter_context(tc.tile_pool(name="op_ps", bufs=2, space="PSUM"))

        NQB = QL // 512  # 2 query blocks of 512
        for qb in range(NQB):
            out2_acc = [
                o2pool.tile([P, S], FP16, tag="o2", name=f"o2_{qb}_{i}")
                for i in range(4)
            ]
            attnT_s = work.tile([P, NHP, 512], FP16, tag="attnT")

            for hp in range(NHP):
                pT = [
                    ptp.tile([P, NKB, 512], FP16, tag=f"pT{h2}", name=f"pT_{qb}_{hp}_{h2}")
                    for h2 in range(2)
                ]
                for qt in range(4):
                    qg = qb * 4 + qt  # global qtile in this core
                    ex = [
                        work.tile([P, S], FP16, tag=f"exp{h2}", name=f"ex_{qb}_{hp}_{qt}_{h2}")
                        for h2 in range(2)
                    ]
                    sums = [
                        small.tile([P, 2], FP32, tag=f"sm{h2}", name=f"sm_{qb}_{hp}_{qt}_{h2}")
                        for h2 in range(2)
                    ]
                    # scores + exp + row-sums, 1024-wide chunks, head pair
                    # interleaved so the K=64 matmuls pack into row groups.
                    for c in range(2):
                        s_ps = [
                            sps.tile([P, 1024], FP32, tag="sps", name=f"sps_{qb}_{hp}_{qt}_{c}_{h2}")
                            for h2 in range(2)
                        ]
                        if "nos" not in ABLATE:
                            for cc in range(2):
                                ks = slice(c * 1024 + cc * 512, c * 1024 + (cc + 1) * 512)
                                for h2 in range(2):
                                    pr = h2 * 64
                                    nc.tensor.matmul(
                                        s_ps[h2][:, cc * 512 : (cc + 1) * 512],
                                        qT_s[pr : pr + 64, hp, qg * P : (qg + 1) * P],
                                        kT_s[pr : pr + 64, hp, ks],
                                        start=True,
                                        stop=True,
                                        tile_position=(pr, 0),
                                    )
                        if "noexp" not in ABLATE:
                            for h2 in range(2):
                                nc.scalar.activation(
                                    ex[h2][:, c * 1024 : (c + 1) * 1024],
                                    s_ps[h2][:],
                                    mybir.ActivationFunctionType.Exp,
                                    scale=float(SM_SCALE),
                                    accum_out=sums[h2][:, c : c + 1],
                                )
                    for h2 in range(2):
                        # r = 1/(16*sum): combine chunk sums, scale, invert
                        sum1 = small.tile([P, 1], FP32, tag="s1", name=f"s1_{qb}_{hp}_{qt}_{h2}")
                        r16 = small.tile([P, 1], FP32, tag="r16", name=f"r16_{qb}_{hp}_{qt}_{h2}")
                        nc.vector.reduce_sum(
                            sum1[:], sums[h2][:], axis=mybir.AxisListType.X
                        )
                        nc.vector.tensor_scalar_mul(sum1[:], sum1[:], 16.0)
                        nc.vector.reciprocal(r16[:], sum1[:])
                        # normalize in place: probs/16 (fp16)
                        nc.vector.tensor_scalar_mul(
                            ex[h2][:], ex[h2][:], r16[:, 0:1]
                        )
                    # mean-probs accumulation (fp16 acc; final cast on DMA out)
                    if "nott" not in ABLATE:
                        if hp == 0:
                            nc.vector.tensor_copy(out2_acc[qt][:], ex[0][:])
                        else:
                            nc.vector.tensor_add(
                                out2_acc[qt][:], out2_acc[qt][:], ex[0][:]
                            )
                        nc.vector.tensor_add(
                            out2_acc[qt][:], out2_acc[qt][:], ex[1][:]
                        )
                    elif hp == 0:
                        nc.vector.tensor_copy(out2_acc[qt][:], ex[0][:])
                    # transpose probs -> [kpos, q] layout for PV
                    if "notrans" not in ABLATE:
                        for h2 in range(2):
                            nc.sync.dma_start_transpose(
                                out=pT[h2][:, :, qt * P : (qt + 1) * P],
                                in_=ex[h2][:],
                            )
                    elif qt == 0:
                        for h2 in range(2):
                            nc.vector.tensor_copy(pT[h2][:, 0, 0:2], ex[h2][:, 0:2])
                # PV: attnT[d, q] accumulated over kpos blocks; col-packed pair.
                # Separate PSUM banks per head: an accumulation group's
                # start=True clears has_written for its whole bank, so two
                # interleaved groups cannot share one bank.
                pv = [
                    pvps.tile([P, 512], FP32, tag=f"pv{h2}", name=f"pv_{qb}_{hp}_{h2}")
                    for h2 in range(2)
                ]
                for kc in range(NKB if "nopv" not in ABLATE else 1):
                    for h2 in range(2):
                        h = hp * 2 + h2
                        pr = h2 * 64
                        nc.tensor.matmul(
                            pv[h2][pr : pr + 64, :],
                            v_s[:, kc, h * 64 : (h + 1) * 64],
                            pT[h2][:, kc, :],
                            start=(kc == 0),
                            stop=(kc == NKB - 1 or "nopv" in ABLATE),
                            tile_position=(0, pr),
                        )
                for h2 in range(2):
                    pr = h2 * 64
                    nc.scalar.copy(
                        attnT_s[pr : pr + 64, hp, :], pv[h2][pr : pr + 64, :]
                    )

            # output projection for this q-block
            for qt in range(4):
                qg = qb * 4 + qt
                ostage = work.tile([P, IN], FP32, tag="ostage", name=f"ost_{qb}_{qt}")
                for oc in range(IN // 512):
                    ops = opps.tile([P, 512], FP32, tag="ops", name=f"ops_{qb}_{qt}_{oc}")
                    for hp in range(NHP):
                        nc.tensor.matmul(
                            ops[:],
                            attnT_s[:, hp, qt * P : (qt + 1) * P],
                            wo_s[:, hp, oc * 512 : (oc + 1) * 512],
                            start=(hp == 0),
                            stop=(hp == NHP - 1),
                        )
                    nc.scalar.copy(ostage[:, oc * 512 : (oc + 1) * 512], ops[:])
                nc.sync.dma_start(
                    out=t["out_sl"][qg * P : (qg + 1) * P, :], in_=ostage[:]
                )
                # SWDGE DMA casts the fp16 accumulator to the f32 output
                nc.gpsimd.dma_start(
                    out=t["out2_sl"][qg * P : (qg + 1) * P, :], in_=out2_acc[qt][:]
                )


_BUILT = {}


def _build(repeats=1):
    key = (repeats, tuple(sorted(ABLATE)))
    if key in _BUILT:
        return _BUILT[key]
    # Bacc (not raw Bass): its compile() pass splits multi-semaphore waits
    # into EventSemaphore pre-waits -- walrus codegen allows only 1 wait per
    # compute instruction.
    nc = bacc.Bacc("TRN2", target_bir_lowering=False, debug=False)
    t = {}
    t["xT"] = nc.dram_tensor("xT", [IN, S], BF16, kind="ExternalInput").ap()
    t["xq"] = nc.dram_tensor("xq", [IN, QL], BF16, kind="ExternalInput").ap()
    t["wq"] = nc.dram_tensor("wq", [IN, IN], BF16, kind="ExternalInput").ap()
    t["wk"] = nc.dram_tensor("wk", [IN, IN], BF16, kind="ExternalInput").ap()
    t["wv"] = nc.dram_tensor("wv", [IN, IN], BF16, kind="ExternalInput").ap()
    t["wo"] = nc.dram_tensor("wo", [IN, IN], FP16, kind="ExternalInput").ap()
    t["bq2"] = nc.dram_tensor("bq2", [P, NIC], FP32, kind="ExternalInput").ap()
    t["bk2"] = nc.dram_tensor("bk2", [P, NIC], FP32, kind="ExternalInput").ap()
    t["out_sl"] = nc.dram_tensor("out_sl", [QL, IN], FP32, kind="ExternalOutput").ap()
    t["out2_sl"] = nc.dram_tensor("out2_sl", [QL, S], FP32, kind="ExternalOutput").ap()

    with tile.TileContext(nc) as tc:
        for _ in range(repeats):  # repeats>1: timing builds only
            _emit(tc, t)
    nc.compile()
    _BUILT[key] = nc
    return nc


def _host_prep(x, Wq, Wk, Wv, Wo, bq, bk):
    """Build the 8 per-core input maps."""
    bf = ml_dtypes.bfloat16
    wq_b = Wq.astype(bf)
    wk_b = Wk.astype(bf)
    wv_b = Wv.astype(bf)
    wo_h = (Wo * np.float32(16.0)).astype(np.float16)
    bq2 = np.ascontiguousarray(bq.reshape(NIC, P).T.astype(np.float32))
    bk2 = np.ascontiguousarray(bk.reshape(NIC, P).T.astype(np.float32))
    in_maps = []
    for b in range(B):
        xT_b = x[b].T.astype(bf)  # [IN, S], contiguous via astype copy
        for half in range(2):
            xq_b = np.ascontiguousarray(xT_b[:, half * QL : (half + 1) * QL])
            in_maps.append(
                {
                    "xT": xT_b,
                    "xq": xq_b,
                    "wq": wq_b,
                    "wk": wk_b,
                    "wv": wv_b,
                    "wo": wo_h,
                    "bq2": bq2,
                    "bk2": bk2,
                }
            )
    return in_maps


def kernel(x, Wq, bq, Wk, bk, Wv, bv, Wo, bo):
    global LAST_RESULTS
    x = np.asarray(x, dtype=np.float32)
    Wq = np.asarray(Wq, dtype=np.float32)
    Wk = np.asarray(Wk, dtype=np.float32)
    Wv = np.asarray(Wv, dtype=np.float32)
    Wo = np.asarray(Wo, dtype=np.float32)
    bq = np.asarray(bq, dtype=np.float32)
    bk = np.asarray(bk, dtype=np.float32)
    bv = np.asarray(bv, dtype=np.float32)
    bo = np.asarray(bo, dtype=np.float32)

    nc = _build()
    in_maps = _host_prep(x, Wq, Wk, Wv, Wo, bq, bk)
    # trace/NTFF profiling is unavailable in this container (no axon.trn
    # hook); run_bass_kernel_spmd with trace=False goes straight to PJRT.
    res = run_bass_kernel_spmd(
        nc, in_maps, core_ids=list(range(NCORES)), trace=False
    )
    LAST_RESULTS = res

    out = np.empty((B, S, IN), dtype=np.float32)
    out2 = np.empty((B, S, S), dtype=np.float32)
    for i in range(NCORES):
        b, half = i // 2, i % 2
        rows = slice(half * QL, (half + 1) * QL)
        out[b, rows, :] = res.results[i]["out_sl"]
        out2[b, rows, :] = res.results[i]["out2_sl"]

    # host bias fixes: probs rows sum to 1 -> attn bias = bv @ Wo; plus bo.
    out += (bv @ Wo + bo)[None, None, :]
    return out, out2
